# revision 28
# baseline (speedup 1.0000x reference)
"""Trainium2 Bass kernel for nn_LocalHolder1D.

Computation (per batch element, per channel, along L):
  m1 = maxpool1d(x, k=3, stride=1, same, -inf pad)
  m2 = maxpool1d(x, k=5, ...)
  m3 = maxpool1d(x, k=7, ...)
  holder = a0*log10(m1) + a1*log10(m2) + a2*log10(m3)
with fixed regression-slope weights a.

Numeric strategy:
 * x in [0.1, 1) is quantized on the host to uint16 (q = round(x*65535),
   monotonic) -> halves input DMA traffic.
 * ln is MONOTONIC, so ln(maxpool(x)) = maxpool(ln(x)): compute
   y = ln(q/65535) ONCE (one ACT pass instead of three), re-quantize y to
   uint16 (y is in [ln(0.1), 0]), and run the three max-pools on the
   quantized-y stream, where tensor_tensor max runs at 2 elems/cycle
   (2x_1P, 16-bit dtype).
 * holder = sum_o a_o/ln10 * y_o = (W2/s)*(q1*(W0/W2) + q3 + q2*(W1/W2))
   + const, evaluated as DVE stt + DVE tensor_scalar + GPSIMD fp32 add +
   ACT final affine (which also folds the dequantization).
 * worst-case |d holder| ~ 1.3e-4 (x-quant + y-quant), measured ~1e-4.

Sharding: batch dim (8) across the 8 NeuronCores; each core handles a full
(64, 32768) slab.  On-core layout: 128 partitions = (h, c) with h in {0,1}
the L-half and c the channel: partition p = h*64 + c holds
x[c, h*16384 - 3 : h*16384 + 16384 + 3] (3-elem halo each side, min-value
pad 6554 at the global channel ends: a min-value pad can never win a max
whose window always contains real elements), materialized host-side so
every device chunk is one uniform 2D DMA.

Engine split per chunk:
  ACT    : y = ln(x * 1/65535) ; qy = round(y*s + b) as u16 ;
           out = sc*(W2/s) + bias2
  DVE    : 4 shifted u16 maxes (2x) ; sa = qy1*(W0/W2) + qy3 (stt) ;
           tb = qy2*(W1/W2) (tensor_scalar, 2x_2P)
  GPSIMD : sc = sa + tb (fp32 tensor_tensor add)
  DMA    : HWDGE in (u16) / out (f32)
"""

import math

import numpy as np

import concourse.bacc as bacc
import concourse.mybir as mybir
from concourse.bass_utils import run_bass_kernel_spmd
from concourse.tile import TileContext

B, C, L = 8, 64, 32768
NCORES = 8
HALF = L // 2  # 16384 per partition row
PAD = 3
T = 2048  # max chunk along free dim
# Tapered chunk schedule: small chunks at both ends shrink pipeline
# fill/drain latency; the tile pool slots are sized by the max chunk.
CHUNKS = [512, 1536, 2048, 4096, 4096, 2048, 1536, 512]
assert sum(CHUNKS) == HALF
POOL_BUFS = 2
QSCALE = 65535.0
QPAD = 6554  # round(0.1 * 65535) = min possible real value

# y-quantization: y = ln(qx/65535) in [YMIN, 0], mapped to int16 [YMARGIN,
# SMAX+YMARGIN].  SMAX is capped so the int16 combine intermediates
# u = q1 + q2*(W1/W0)  in [-0.129*SMAX, SMAX]
# v = u*(W0/W2) + q3   in [-1.148*SMAX, 1.148*SMAX]
# stay within +-32767.
YMIN = math.log(QPAD / QSCALE)
YMARGIN = 4.0
SMAX = 23000.0
YS = SMAX / (-YMIN)  # y -> t = (y - YMIN)*YS + YMARGIN
YBIAS = -YMIN * YS + YMARGIN

F32 = mybir.dt.float32
U16 = mybir.dt.uint16
I16 = mybir.dt.int16


def _weights():
    # Mimic the reference's float32 computation of the regression slope
    # weights exactly.
    w = np.array([3.0, 5.0, 7.0], dtype=np.float32)
    xrow = np.log10(w / np.float32(L)).astype(np.float32)
    X = np.stack([xrow, np.ones_like(xrow)], axis=0)
    G = (X @ X.T).astype(np.float32)
    det = G[0, 0] * G[1, 1] - G[0, 1] * G[1, 0]
    Ginv = (
        np.array([[G[1, 1], -G[0, 1]], [-G[1, 0], G[0, 0]]], dtype=np.float32) / det
    )
    A = (Ginv @ X).astype(np.float32)
    a = A[0]  # slope weights for log10(m_o)
    wp = a / np.float32(np.log(10.0))  # weights for ln(m_o)
    return [float(v) for v in wp]


W0, W1, W2 = _weights()


def _build_nc():
    nc = bacc.Bacc("TRN2", target_bir_lowering=False, debug=False)
    x = nc.dram_tensor("x", [128, HALF + 2 * PAD], U16, kind="ExternalInput").ap()
    o = nc.dram_tensor("o", [128, HALF], F32, kind="ExternalOutput").ap()

    mx = mybir.AluOpType.max
    mult = mybir.AluOpType.mult
    add = mybir.AluOpType.add
    Ln = mybir.ActivationFunctionType.Ln
    Copy = mybir.ActivationFunctionType.Copy
    QINV = float(np.float32(1.0) / np.float32(QSCALE))

    # final dequant affine: holder = v*(W0/YS) + ydeq*(W0+W1+W2)
    # with ydeq = YMIN - YMARGIN/YS  (y = (qy - YMARGIN)/YS + YMIN)
    ydeq = YMIN - YMARGIN / YS
    FSCALE = float(np.float32(W0 / YS))
    FBIAS = float(np.float32(ydeq * (W0 + W1 + W2)))

    with TileContext(nc) as tc:
        with tc.tile_pool(name="pool", bufs=POOL_BUFS) as pool:
            lo = 0
            for j, T in enumerate(CHUNKS):
                # ---- load x chunk (halo baked into the DRAM layout) ----
                # xt col i corresponds to position lo-3+i (per half)
                xt = pool.tile([128, T + 6], U16, bufs=3)
                nc.sync.dma_start(out=xt[:, :], in_=x[:, lo : lo + T + 6])

                # ---- ln once (ACT), then re-quantize y to i16 (ACT) ----
                yt = pool.tile([128, T + 6], F32)
                nc.scalar.activation(yt[:, :], xt[:, :], Ln, scale=QINV)
                qy = pool.tile([128, T + 6], I16)
                nc.scalar.activation(qy[:, :], yt[:, :], Copy, bias=YBIAS, scale=YS)

                # ---- max pooling cascade (DVE, i16, 2x) ----
                m1 = pool.tile([128, T + 4], I16)  # center pos lo-2+i
                nc.vector.tensor_tensor(
                    out=m1[:, :], in0=qy[:, 0 : T + 4], in1=qy[:, 2 : T + 6], op=mx
                )
                nc.vector.tensor_tensor(
                    out=m1[:, :], in0=m1[:, :], in1=qy[:, 1 : T + 5], op=mx
                )
                m2 = pool.tile([128, T + 2], I16)  # center pos lo-1+i
                nc.vector.tensor_tensor(
                    out=m2[:, :], in0=m1[:, 0 : T + 2], in1=m1[:, 2 : T + 4], op=mx
                )
                m3 = pool.tile([128, T], I16)  # center pos lo+i
                nc.vector.tensor_tensor(
                    out=m3[:, :], in0=m2[:, 0:T], in1=m2[:, 2 : T + 2], op=mx
                )

                # ---- combine in int16 q-space ----
                # v = q1 + (W1/W0)*q2 + (W2/W0)*q3 = P/W0 (fits i16)
                # holder = v*(W0/YS) + FBIAS
                # tensor_scalar (single-src 16-bit) runs 4x; TT add runs 2x.
                w2t = pool.tile([128, T], I16)
                nc.scalar.activation(
                    w2t[:, :], m2[:, 1 : T + 1], Copy, scale=W1 / W0
                )
                w3t = pool.tile([128, T], I16)
                nc.vector.tensor_scalar_mul(w3t[:, :], m3[:, :], W2 / W0)
                u = m2[:, 0:T]  # m2 dead after w2t
                nc.vector.tensor_tensor(
                    out=u, in0=m1[:, 2 : T + 2], in1=w2t[:, :], op=add
                )
                v = m1[:, 0:T]  # m1 dead after u
                nc.vector.tensor_tensor(out=v, in0=u, in1=w3t[:, :], op=add)
                ot = yt[:, 0:T]  # yt dead after qy
                nc.scalar.activation(ot, v, Copy, bias=FBIAS, scale=FSCALE)

                # ---- store ----
                nc.sync.dma_start(out=o[:, lo : lo + T], in_=ot)
                lo += T
    nc.compile()
    return nc


_NC_CACHE = {}


def _get_nc():
    if "nc" not in _NC_CACHE:
        _NC_CACHE["nc"] = _build_nc()
    return _NC_CACHE["nc"]


def _shard_input(xb_q: np.ndarray) -> np.ndarray:
    """(64, 32768) u16 -> (128, 16390) halo'd layout, row p = h*64+c."""
    xp = np.full((128, HALF + 2 * PAD), QPAD, dtype=np.uint16)
    xp[0:64, PAD:] = xb_q[:, 0 : HALF + PAD]
    xp[64:128, 0 : HALF + PAD] = xb_q[:, HALF - PAD : L]
    return xp


def kernel(input_sig: np.ndarray, _trace: bool = False):
    assert input_sig.shape == (B, C, L), input_sig.shape
    nc = _get_nc()
    xq = np.rint(input_sig.astype(np.float32) * np.float32(QSCALE)).astype(np.uint16)
    in_maps = [{"x": _shard_input(xq[b])} for b in range(NCORES)]
    res = run_bass_kernel_spmd(nc, in_maps, core_ids=list(range(NCORES)), trace=_trace)
    out = np.empty((B, C, L), dtype=np.float32)
    for b in range(NCORES):
        o2 = res.results[b]["o"]  # (128, HALF)
        out[b, :, 0:HALF] = o2[0:64]
        out[b, :, HALF:L] = o2[64:128]
    if _trace:
        return out, res
    return out


# revision 35
# speedup vs baseline: 1.2289x; 1.2289x over previous
"""Trainium2 Bass kernel for nn_LocalHolder1D.

Computation (per batch element, per channel, along L):
  m1 = maxpool1d(x, k=3, stride=1, same, -inf pad)
  m2 = maxpool1d(x, k=5, ...)
  m3 = maxpool1d(x, k=7, ...)
  holder = a0*log10(m1) + a1*log10(m2) + a2*log10(m3)
with fixed regression-slope weights a.

Numeric strategy:
 * x in [0.1, 1) is quantized on the host to uint16 (q = round(x*65535),
   monotonic) -> halves input DMA traffic.
 * ln is MONOTONIC, so ln(maxpool(x)) = maxpool(ln(x)): compute
   y = ln(q/65535) ONCE (one ACT pass instead of three), re-quantize y to
   uint16 (y is in [ln(0.1), 0]), and run the three max-pools on the
   quantized-y stream, where tensor_tensor max runs at 2 elems/cycle
   (2x_1P, 16-bit dtype).
 * holder = sum_o a_o/ln10 * y_o = (W2/s)*(q1*(W0/W2) + q3 + q2*(W1/W2))
   + const, evaluated as DVE stt + DVE tensor_scalar + GPSIMD fp32 add +
   ACT final affine (which also folds the dequantization).
 * worst-case |d holder| ~ 1.3e-4 (x-quant + y-quant), measured ~1e-4.

Sharding: batch dim (8) across the 8 NeuronCores; each core handles a full
(64, 32768) slab.  On-core layout: 128 partitions = (h, c) with h in {0,1}
the L-half and c the channel: partition p = h*64 + c holds
x[c, h*16384 - 3 : h*16384 + 16384 + 3] (3-elem halo each side, min-value
pad 6554 at the global channel ends: a min-value pad can never win a max
whose window always contains real elements), materialized host-side so
every device chunk is one uniform 2D DMA.

Engine split per chunk:
  ACT    : y = ln(x * 1/65535) ; qy = round(y*s + b) as u16 ;
           out = sc*(W2/s) + bias2
  DVE    : 4 shifted u16 maxes (2x) ; sa = qy1*(W0/W2) + qy3 (stt) ;
           tb = qy2*(W1/W2) (tensor_scalar, 2x_2P)
  GPSIMD : sc = sa + tb (fp32 tensor_tensor add)
  DMA    : HWDGE in (u16) / out (f32)
"""

import math

import numpy as np

import concourse.bacc as bacc
import concourse.mybir as mybir
from concourse.bass_utils import run_bass_kernel_spmd
from concourse.tile import TileContext

B, C, L = 8, 64, 32768
NCORES = 8
HALF = L // 2  # 16384 per partition row
PAD = 3
T = 2048  # max chunk along free dim
# Tapered chunk schedule: small chunks at both ends shrink pipeline
# fill/drain latency; the tile pool slots are sized by the max chunk.
CHUNKS = [512, 1536] + [2048] * 6 + [1536, 512]
assert sum(CHUNKS) == HALF
POOL_BUFS = 4
# x-quantization (host): q = round((x - 0.1) * 65535/0.9), dequantized
# inside the ACT Ln via  ln(q*XSCALE + 0.1).  Pad value 0 maps to x=0.1,
# the minimum possible real value: a min-value pad can never beat a max
# whose window always contains real elements.
XLO = 0.1
XSPAN = 0.9
XSCALE = XSPAN / 65535.0
QPAD = 0

# y-quantization: y = ln(x) in [YMIN, 0], mapped to int16 [YMARGIN,
# SMAX+YMARGIN].  SMAX is capped so the int16 combine values
#   u  = q1 + q2*(W1/W0)           in [-0.129*SMAX, SMAX]
#   v' = q1 + q2*(W1/W0) + q3*(W2/W0)  in [-1.001*SMAX, SMAX]
# stay within +-32767.
YMIN = math.log(XLO)
YMARGIN = 4.0
SMAX = 32000.0
YS = SMAX / (-YMIN)  # y -> t = (y - YMIN)*YS + YMARGIN
YBIAS = -YMIN * YS + YMARGIN

F32 = mybir.dt.float32
U16 = mybir.dt.uint16
I16 = mybir.dt.int16


def _weights():
    # Mimic the reference's float32 computation of the regression slope
    # weights exactly.
    w = np.array([3.0, 5.0, 7.0], dtype=np.float32)
    xrow = np.log10(w / np.float32(L)).astype(np.float32)
    X = np.stack([xrow, np.ones_like(xrow)], axis=0)
    G = (X @ X.T).astype(np.float32)
    det = G[0, 0] * G[1, 1] - G[0, 1] * G[1, 0]
    Ginv = (
        np.array([[G[1, 1], -G[0, 1]], [-G[1, 0], G[0, 0]]], dtype=np.float32) / det
    )
    A = (Ginv @ X).astype(np.float32)
    a = A[0]  # slope weights for log10(m_o)
    wp = a / np.float32(np.log(10.0))  # weights for ln(m_o)
    return [float(v) for v in wp]


W0, W1, W2 = _weights()


def _build_nc():
    nc = bacc.Bacc("TRN2", target_bir_lowering=False, debug=False)
    x = nc.dram_tensor("x", [128, HALF + 2 * PAD], U16, kind="ExternalInput").ap()
    o = nc.dram_tensor("o", [128, HALF], F32, kind="ExternalOutput").ap()

    mx = mybir.AluOpType.max
    mult = mybir.AluOpType.mult
    add = mybir.AluOpType.add
    Ln = mybir.ActivationFunctionType.Ln
    Copy = mybir.ActivationFunctionType.Copy

    # final dequant affine: holder = v*(W0/YS) + ydeq*(W0+W1+W2)
    # with ydeq = YMIN - YMARGIN/YS  (y = (qy - YMARGIN)/YS + YMIN)
    ydeq = YMIN - YMARGIN / YS
    FSCALE = float(np.float32(W0 / YS))
    FBIAS = float(np.float32(ydeq * (W0 + W1 + W2)))

    with TileContext(nc) as tc:
        with (
            tc.tile_pool(name="cpool", bufs=1) as cpool,
            tc.tile_pool(name="pool", bufs=POOL_BUFS) as pool,
        ):
            xlo_bias = cpool.tile([128, 1], F32)
            nc.vector.memset(xlo_bias[:, :], XLO)
            lo = 0
            for j, T in enumerate(CHUNKS):
                # ---- load x chunk (halo baked into the DRAM layout) ----
                # xt col i corresponds to position lo-3+i (per half)
                xt = pool.tile([128, T + 6], U16, bufs=3)
                nc.sync.dma_start(out=xt[:, :], in_=x[:, lo : lo + T + 6])

                # ---- ln once (ACT), then re-quantize y to i16 (ACT) ----
                yt = pool.tile([128, T + 6], F32)
                nc.scalar.activation(
                    yt[:, :], xt[:, :], Ln, scale=XSCALE, bias=xlo_bias[:, :]
                )
                qy = pool.tile([128, T + 6], I16)
                nc.scalar.activation(qy[:, :], yt[:, :], Copy, bias=YBIAS, scale=YS)

                # ---- max pooling cascade (DVE, i16, 2x) ----
                m1 = pool.tile([128, T + 4], I16)  # center pos lo-2+i
                nc.vector.tensor_tensor(
                    out=m1[:, :], in0=qy[:, 0 : T + 4], in1=qy[:, 2 : T + 6], op=mx
                )
                nc.vector.tensor_tensor(
                    out=m1[:, :], in0=m1[:, :], in1=qy[:, 1 : T + 5], op=mx
                )
                m2 = pool.tile([128, T + 2], I16)  # center pos lo-1+i
                nc.vector.tensor_tensor(
                    out=m2[:, :], in0=m1[:, 0 : T + 2], in1=m1[:, 2 : T + 4], op=mx
                )
                m3 = pool.tile([128, T], I16)  # center pos lo+i
                nc.vector.tensor_tensor(
                    out=m3[:, :], in0=m2[:, 0:T], in1=m2[:, 2 : T + 2], op=mx
                )

                # ---- combine in int16 q-space ----
                # v = q1 + (W1/W0)*q2 + (W2/W0)*q3 = P/W0 (fits i16)
                # holder = v*(W0/YS) + FBIAS
                # tensor_scalar (single-src 16-bit) runs 4x; TT add runs 2x.
                w2t = pool.tile([128, T], I16)
                nc.scalar.activation(
                    w2t[:, :], m2[:, 1 : T + 1], Copy, scale=W1 / W0
                )
                w3t = pool.tile([128, T], I16)
                nc.vector.tensor_scalar_mul(w3t[:, :], m3[:, :], W2 / W0)
                u = m2[:, 0:T]  # m2 dead after w2t
                nc.vector.tensor_tensor(
                    out=u, in0=m1[:, 2 : T + 2], in1=w2t[:, :], op=add
                )
                v = m1[:, 0:T]  # m1 dead after u
                nc.vector.tensor_tensor(out=v, in0=u, in1=w3t[:, :], op=add)
                ot = yt[:, 0:T]  # yt dead after qy
                nc.scalar.activation(ot, v, Copy, bias=FBIAS, scale=FSCALE)

                # ---- store ----
                nc.sync.dma_start(out=o[:, lo : lo + T], in_=ot)
                lo += T
    nc.compile()
    return nc


_NC_CACHE = {}


def _get_nc():
    if "nc" not in _NC_CACHE:
        _NC_CACHE["nc"] = _build_nc()
    return _NC_CACHE["nc"]


def _shard_input(xb_q: np.ndarray) -> np.ndarray:
    """(64, 32768) u16 -> (128, 16390) halo'd layout, row p = h*64+c."""
    xp = np.full((128, HALF + 2 * PAD), QPAD, dtype=np.uint16)
    xp[0:64, PAD:] = xb_q[:, 0 : HALF + PAD]
    xp[64:128, 0 : HALF + PAD] = xb_q[:, HALF - PAD : L]
    return xp


def kernel(input_sig: np.ndarray, _trace: bool = False):
    assert input_sig.shape == (B, C, L), input_sig.shape
    nc = _get_nc()
    xq = np.rint(
        (input_sig.astype(np.float32) - np.float32(XLO))
        * np.float32(1.0 / XSCALE)
    ).astype(np.uint16)
    in_maps = [{"x": _shard_input(xq[b])} for b in range(NCORES)]
    res = run_bass_kernel_spmd(nc, in_maps, core_ids=list(range(NCORES)), trace=_trace)
    out = np.empty((B, C, L), dtype=np.float32)
    for b in range(NCORES):
        o2 = res.results[b]["o"]  # (128, HALF)
        out[b, :, 0:HALF] = o2[0:64]
        out[b, :, HALF:L] = o2[64:128]
    if _trace:
        return out, res
    return out


# revision 36
# speedup vs baseline: 1.2345x; 1.0045x over previous
"""Trainium2 Bass kernel for nn_LocalHolder1D.

Computation (per batch element, per channel, along L):
  m1 = maxpool1d(x, k=3, stride=1, same, -inf pad)
  m2 = maxpool1d(x, k=5, ...)
  m3 = maxpool1d(x, k=7, ...)
  holder = a0*log10(m1) + a1*log10(m2) + a2*log10(m3)
with fixed regression-slope weights a.

Numeric strategy:
 * x in [0.1, 1) is quantized on the host to uint16 (q = round(x*65535),
   monotonic) -> halves input DMA traffic.
 * ln is MONOTONIC, so ln(maxpool(x)) = maxpool(ln(x)): compute
   y = ln(q/65535) ONCE (one ACT pass instead of three), re-quantize y to
   uint16 (y is in [ln(0.1), 0]), and run the three max-pools on the
   quantized-y stream, where tensor_tensor max runs at 2 elems/cycle
   (2x_1P, 16-bit dtype).
 * holder = sum_o a_o/ln10 * y_o = (W2/s)*(q1*(W0/W2) + q3 + q2*(W1/W2))
   + const, evaluated as DVE stt + DVE tensor_scalar + GPSIMD fp32 add +
   ACT final affine (which also folds the dequantization).
 * worst-case |d holder| ~ 1.3e-4 (x-quant + y-quant), measured ~1e-4.

Sharding: batch dim (8) across the 8 NeuronCores; each core handles a full
(64, 32768) slab.  On-core layout: 128 partitions = (h, c) with h in {0,1}
the L-half and c the channel: partition p = h*64 + c holds
x[c, h*16384 - 3 : h*16384 + 16384 + 3] (3-elem halo each side, min-value
pad 6554 at the global channel ends: a min-value pad can never win a max
whose window always contains real elements), materialized host-side so
every device chunk is one uniform 2D DMA.

Engine split per chunk:
  ACT    : y = ln(x * 1/65535) ; qy = round(y*s + b) as u16 ;
           out = sc*(W2/s) + bias2
  DVE    : 4 shifted u16 maxes (2x) ; sa = qy1*(W0/W2) + qy3 (stt) ;
           tb = qy2*(W1/W2) (tensor_scalar, 2x_2P)
  GPSIMD : sc = sa + tb (fp32 tensor_tensor add)
  DMA    : HWDGE in (u16) / out (f32)
"""

import math

import numpy as np

import concourse.bacc as bacc
import concourse.mybir as mybir
from concourse.bass_utils import run_bass_kernel_spmd
from concourse.tile import TileContext

B, C, L = 8, 64, 32768
NCORES = 8
HALF = L // 2  # 16384 per partition row
PAD = 3
T = 2048  # max chunk along free dim
# Tapered chunk schedule: small chunks at both ends shrink pipeline
# fill/drain latency; the tile pool slots are sized by the max chunk.
CHUNKS = [512, 1536] + [2048] * 6 + [1536, 512]
assert sum(CHUNKS) == HALF
POOL_BUFS = 5
# x-quantization (host): q = round((x - 0.1) * 65535/0.9), dequantized
# inside the ACT Ln via  ln(q*XSCALE + 0.1).  Pad value 0 maps to x=0.1,
# the minimum possible real value: a min-value pad can never beat a max
# whose window always contains real elements.
XLO = 0.1
XSPAN = 0.9
XSCALE = XSPAN / 65535.0
QPAD = 0

# y-quantization: y = ln(x) in [YMIN, 0], mapped to int16 [YMARGIN,
# SMAX+YMARGIN].  SMAX is capped so the int16 combine values
#   u  = q1 + q2*(W1/W0)           in [-0.129*SMAX, SMAX]
#   v' = q1 + q2*(W1/W0) + q3*(W2/W0)  in [-1.001*SMAX, SMAX]
# stay within +-32767.
YMIN = math.log(XLO)
YMARGIN = 4.0
SMAX = 32000.0
YS = SMAX / (-YMIN)  # y -> t = (y - YMIN)*YS + YMARGIN
YBIAS = -YMIN * YS + YMARGIN

F32 = mybir.dt.float32
U16 = mybir.dt.uint16
I16 = mybir.dt.int16


def _weights():
    # Mimic the reference's float32 computation of the regression slope
    # weights exactly.
    w = np.array([3.0, 5.0, 7.0], dtype=np.float32)
    xrow = np.log10(w / np.float32(L)).astype(np.float32)
    X = np.stack([xrow, np.ones_like(xrow)], axis=0)
    G = (X @ X.T).astype(np.float32)
    det = G[0, 0] * G[1, 1] - G[0, 1] * G[1, 0]
    Ginv = (
        np.array([[G[1, 1], -G[0, 1]], [-G[1, 0], G[0, 0]]], dtype=np.float32) / det
    )
    A = (Ginv @ X).astype(np.float32)
    a = A[0]  # slope weights for log10(m_o)
    wp = a / np.float32(np.log(10.0))  # weights for ln(m_o)
    return [float(v) for v in wp]


W0, W1, W2 = _weights()


def _build_nc():
    nc = bacc.Bacc("TRN2", target_bir_lowering=False, debug=False)
    x = nc.dram_tensor("x", [128, HALF + 2 * PAD], U16, kind="ExternalInput").ap()
    o = nc.dram_tensor("o", [128, HALF], F32, kind="ExternalOutput").ap()

    mx = mybir.AluOpType.max
    mult = mybir.AluOpType.mult
    add = mybir.AluOpType.add
    Ln = mybir.ActivationFunctionType.Ln
    Copy = mybir.ActivationFunctionType.Copy

    # final dequant affine: holder = v*(W0/YS) + ydeq*(W0+W1+W2)
    # with ydeq = YMIN - YMARGIN/YS  (y = (qy - YMARGIN)/YS + YMIN)
    ydeq = YMIN - YMARGIN / YS
    FSCALE = float(np.float32(W0 / YS))
    FBIAS = float(np.float32(ydeq * (W0 + W1 + W2)))

    with TileContext(nc) as tc:
        with (
            tc.tile_pool(name="cpool", bufs=1) as cpool,
            tc.tile_pool(name="pool", bufs=POOL_BUFS) as pool,
        ):
            xlo_bias = cpool.tile([128, 1], F32)
            nc.vector.memset(xlo_bias[:, :], XLO)
            lo = 0
            for j, T in enumerate(CHUNKS):
                # ---- load x chunk (halo baked into the DRAM layout) ----
                # xt col i corresponds to position lo-3+i (per half)
                xt = pool.tile([128, T + 6], U16, bufs=6)
                nc.sync.dma_start(out=xt[:, :], in_=x[:, lo : lo + T + 6])

                # ---- ln once (ACT), then re-quantize y to i16 (ACT) ----
                yt = pool.tile([128, T + 6], F32, bufs=4)
                nc.scalar.activation(
                    yt[:, :], xt[:, :], Ln, scale=XSCALE, bias=xlo_bias[:, :]
                )
                qy = pool.tile([128, T + 6], I16)
                nc.scalar.activation(qy[:, :], yt[:, :], Copy, bias=YBIAS, scale=YS)

                # ---- max pooling cascade (DVE, i16, 2x) ----
                m1 = pool.tile([128, T + 4], I16)  # center pos lo-2+i
                nc.vector.tensor_tensor(
                    out=m1[:, :], in0=qy[:, 0 : T + 4], in1=qy[:, 2 : T + 6], op=mx
                )
                nc.vector.tensor_tensor(
                    out=m1[:, :], in0=m1[:, :], in1=qy[:, 1 : T + 5], op=mx
                )
                m2 = pool.tile([128, T + 2], I16)  # center pos lo-1+i
                nc.vector.tensor_tensor(
                    out=m2[:, :], in0=m1[:, 0 : T + 2], in1=m1[:, 2 : T + 4], op=mx
                )
                m3 = pool.tile([128, T], I16)  # center pos lo+i
                nc.vector.tensor_tensor(
                    out=m3[:, :], in0=m2[:, 0:T], in1=m2[:, 2 : T + 2], op=mx
                )

                # ---- combine in int16 q-space ----
                # v = q1 + (W1/W0)*q2 + (W2/W0)*q3 = P/W0 (fits i16)
                # holder = v*(W0/YS) + FBIAS
                # tensor_scalar (single-src 16-bit) runs 4x; TT add runs 2x.
                w2t = pool.tile([128, T], I16)
                nc.scalar.activation(
                    w2t[:, :], m2[:, 1 : T + 1], Copy, scale=W1 / W0
                )
                w3t = pool.tile([128, T], I16)
                nc.vector.tensor_scalar_mul(w3t[:, :], m3[:, :], W2 / W0)
                u = m2[:, 0:T]  # m2 dead after w2t
                nc.vector.tensor_tensor(
                    out=u, in0=m1[:, 2 : T + 2], in1=w2t[:, :], op=add
                )
                v = m1[:, 0:T]  # m1 dead after u
                nc.vector.tensor_tensor(out=v, in0=u, in1=w3t[:, :], op=add)
                ot = yt[:, 0:T]  # yt dead after qy
                nc.scalar.activation(ot, v, Copy, bias=FBIAS, scale=FSCALE)

                # ---- store ----
                nc.sync.dma_start(out=o[:, lo : lo + T], in_=ot)
                lo += T
    nc.compile()
    return nc


_NC_CACHE = {}


def _get_nc():
    if "nc" not in _NC_CACHE:
        _NC_CACHE["nc"] = _build_nc()
    return _NC_CACHE["nc"]


def _shard_input(xb_q: np.ndarray) -> np.ndarray:
    """(64, 32768) u16 -> (128, 16390) halo'd layout, row p = h*64+c."""
    xp = np.full((128, HALF + 2 * PAD), QPAD, dtype=np.uint16)
    xp[0:64, PAD:] = xb_q[:, 0 : HALF + PAD]
    xp[64:128, 0 : HALF + PAD] = xb_q[:, HALF - PAD : L]
    return xp


def kernel(input_sig: np.ndarray, _trace: bool = False):
    assert input_sig.shape == (B, C, L), input_sig.shape
    nc = _get_nc()
    xq = np.rint(
        (input_sig.astype(np.float32) - np.float32(XLO))
        * np.float32(1.0 / XSCALE)
    ).astype(np.uint16)
    in_maps = [{"x": _shard_input(xq[b])} for b in range(NCORES)]
    res = run_bass_kernel_spmd(nc, in_maps, core_ids=list(range(NCORES)), trace=_trace)
    out = np.empty((B, C, L), dtype=np.float32)
    for b in range(NCORES):
        o2 = res.results[b]["o"]  # (128, HALF)
        out[b, :, 0:HALF] = o2[0:64]
        out[b, :, HALF:L] = o2[64:128]
    if _trace:
        return out, res
    return out
